# revision 13
# baseline (speedup 1.0000x reference)
"""2-layer GAT (PyG semantics) on 8 Trainium2 NeuronCores via Bass/Tile.

v2 design:
  - dst-sharded edges with degree-balanced node->(core,block) assignment
    (host permutation, inverse-applied to the output).
  - pass0: h1|s1|d1 = x @ Wcat1 from host-pretransposed f16 x; rows are
    h-major 768B: [256 h f16 | 4 s f16 | 4 d f16 | pad]. d stashed in SBUF.
  - AllGather split into 4 row-chunks per layer (Shared outputs), issued as
    soon as the producing blocks are done, so transfers overlap compute.
  - edge pass per chunk-of-4-blocks: ONE dma_gather per (chunk, src-subrange)
    (amortizes the ~1us fixed SWDGE cost), one-hot D/DT streamed from DRAM in
    a host-pretransposed layout (single contiguous DMA per chunk).
  - attention: dxp = D @ dblk on PE; e = s+dxp; lrelu via fused
    tensor_scalar(min,*-0.8)+add on DVE (scalar engine runs Exp ONLY -> no
    activation-table thrash); p=exp(e); M = [G*p | p]; agg += DT^T @ M
    accumulated in PSUM across all 4 src-subranges of a block.
  - log-softmax batched once over all blocks at the end (single Ln).
"""
import sys
sys.path.insert(0, "/opt/trn_rl_repo")
import numpy as np

import concourse.bass as bass
import concourse.bacc as bacc
import concourse.tile as tile
from concourse import mybir, library_config
from concourse import bass_utils

P = 128
NEG_SLOPE = 0.2
N_NODES = 100000
FIN = 165
HID = 64
H1 = 4
NC_OUT = 2
NCORES = 8
NPC = N_NODES // NCORES          # 12500
NBLK = (NPC + P - 1) // P        # 98
NSUB = 4                         # AllGather row-chunks / gather tables
SUBLEN = [3200, 3200, 3200, 2900]
SUBB = [25, 25, 25, 23]          # blocks per sub
RU1 = 384                        # f16 units per L1 row (768B)
RU2 = 128                        # f16 units per L2 row (256B)
CHUNK = 4                        # blocks per gather call
GM = 8                           # max tiles per (block, sub) matmul group

f16 = mybir.dt.float16
f32 = mybir.dt.float32
i16 = mybir.dt.int16
AF = mybir.ActivationFunctionType
ALU = mybir.AluOpType
AXX = mybir.AxisListType.X


def _host_assign(edge_index):
    """Degree-balanced node -> (core, block, slot) assignment."""
    src = np.concatenate([np.asarray(edge_index[0]), np.arange(N_NODES)]).astype(np.int64)
    dst = np.concatenate([np.asarray(edge_index[1]), np.arange(N_NODES)]).astype(np.int64)
    deg = np.bincount(dst, minlength=N_NODES)
    order = np.argsort(-deg, kind="stable")
    # groups of 8 consecutive (similar-degree) nodes -> one per core; within a
    # core, deal degree ranks round-robin across blocks so every block has the
    # same degree profile (last block has only 84 slots).
    ranked = order.reshape(NPC, NCORES).T            # [NCORES, NPC] by rank
    r = np.arange(NPC)
    full = 84 * NBLK                                 # rounds covering all blocks
    blk = np.where(r < full, r % NBLK, (r - full) % (NBLK - 1))
    slot = np.where(r < full, r // NBLK, 84 + (r - full) // (NBLK - 1))
    lpos_of_rank = blk * P + slot
    perm = np.empty((NCORES, NPC), np.int64)
    for k in range(NCORES):
        perm[k][lpos_of_rank] = ranked[k]
    # node -> (core, localpos)
    core_of = np.empty(N_NODES, np.int64)
    lpos_of = np.empty(N_NODES, np.int64)
    for k in range(NCORES):
        core_of[perm[k]] = k
        lpos_of[perm[k]] = np.arange(NPC)
    return src, dst, perm, core_of, lpos_of


def _host_prep(edge_index):
    src, dst, perm, core_of, lpos_of = _host_assign(edge_index)
    sub_starts = np.array([0, 3200, 6400, 9600, 12500])
    # table position of a (global) node, per-sub tables of [8*len_s] rows
    l_all = lpos_of
    s_all = np.minimum(l_all // 3200, 3)
    len_all = np.array(SUBLEN)[s_all]
    pos_all = core_of * len_all + (l_all - sub_starts[s_all])

    dcore = core_of[dst]
    dl = lpos_of[dst]
    blk_all = dl // P
    slot_all = dl % P

    # per (core, block, sub) counts
    cnt = np.zeros((NCORES, NBLK, NSUB), np.int64)
    percore = []
    for k in range(NCORES):
        sel = np.nonzero(dcore == k)[0]
        e_pos = pos_all[src[sel]]
        e_sub = s_all[src[sel]]
        e_blk = blk_all[sel]
        e_slot = slot_all[sel]
        o = np.lexsort((e_pos, e_sub, e_blk))
        percore.append((e_pos[o], e_sub[o], e_blk[o], e_slot[o]))
        np.add.at(cnt[k], (e_blk, e_sub), 1)
    reg = np.maximum(cnt.max(axis=0), 1)
    tiles_bs = (reg + P - 1) // P                     # [NBLK, NSUB]
    assert tiles_bs.max() <= GM, tiles_bs.max()

    # chunk structure: blocks [4c, 4c+4)
    nchunk = (NBLK + CHUNK - 1) // CHUNK
    # tile order: for c: for s: for b in chunk
    t_of = {}
    t0 = 0
    calls = []                                        # (c, s, t0, T)
    for c in range(nchunk):
        bs = range(c * CHUNK, min((c + 1) * CHUNK, NBLK))
        for s in range(NSUB):
            call_t0 = t0
            for b in bs:
                t_of[(b, s)] = t0
                t0 += int(tiles_bs[b, s])
            calls.append((c, s, call_t0, t0 - call_t0))
    ntiles = t0
    plan = dict(tiles_bs=tiles_bs, ntiles=ntiles, calls=calls, nchunk=nchunk)

    data = []
    for k in range(NCORES):
        e_pos, e_sub, e_blk, e_slot = percore[k]
        flat_pos = np.zeros(ntiles * P, np.int64)     # pad pos = 0 (valid row)
        flat_slot = np.full(ntiles * P, -1, np.int64)
        # bucket boundaries per (b, s)
        keys = e_blk * NSUB + e_sub
        bounds = np.searchsorted(keys, np.arange(NBLK * NSUB + 1))
        for b in range(NBLK):
            for s in range(NSUB):
                lo, hi = bounds[b * NSUB + s], bounds[b * NSUB + s + 1]
                L = hi - lo
                if L == 0:
                    continue
                base = t_of[(b, s)] * P
                flat_pos[base:base + L] = e_pos[lo:hi]
                flat_slot[base:base + L] = e_slot[lo:hi]
        # idx stream: wrap 16 per CALL region, then replicate to 128 chans
        idx16 = np.zeros((16, ntiles * P // 16), np.int16)
        for (c, s, ct0, T) in calls:
            if T == 0:
                continue
            n = T * P
            seg = flat_pos[ct0 * P: ct0 * P + n]
            assert seg.max() < 32768
            idx16[:, ct0 * 8:(ct0 + T) * 8] = \
                seg.reshape(n // 16, 16).T.astype(np.int16)
        idx_stream = np.tile(idx16, (8, 1))           # [128, ntiles*8]

        # ddT [128, ntiles*256]: cols t*256+i  = DT[e=part, i]  (i=slot)
        #                        cols t*256+128+e = D[i=part, e]
        ddT = np.zeros((P, ntiles * 256), np.float16)
        t_idx = np.repeat(np.arange(ntiles), P)
        e_idx = np.tile(np.arange(P), ntiles)
        v = flat_slot >= 0
        ddT[e_idx[v], t_idx[v] * 256 + flat_slot[v]] = 1.0
        ddT[flat_slot[v], t_idx[v] * 256 + 128 + e_idx[v]] = 1.0
        data.append(dict(idx_stream=idx_stream, ddT=ddT))
    return plan, data, perm


def _host_weights(W1, a_src1, a_dst1, W2, a_src2, a_dst2):
    W1 = np.asarray(W1, np.float32); W2 = np.asarray(W2, np.float32)
    a_src1 = np.asarray(a_src1, np.float32); a_dst1 = np.asarray(a_dst1, np.float32)
    a_src2 = np.asarray(a_src2, np.float32); a_dst2 = np.asarray(a_dst2, np.float32)
    Wcat1 = np.zeros((FIN, 264), np.float32)
    Wcat1[:, 0:256] = W1
    for h in range(H1):
        Wcat1[:, 256 + h] = W1[:, h * HID:(h + 1) * HID] @ a_src1[h]
        Wcat1[:, 260 + h] = W1[:, h * HID:(h + 1) * HID] @ a_dst1[h]
    Wcat2 = np.zeros((H1 * HID, 66), np.float32)
    Wcat2[:, :HID] = W2
    Wcat2[:, 64] = W2 @ a_src2[0]
    Wcat2[:, 65] = W2 @ a_dst2[0]
    return Wcat1.astype(np.float16), Wcat2.astype(np.float16)


def _host_xt(x, perm_k):
    """[128, NBLK*256] f16: xT[p, b*256+g*128+n] = x[perm[b*128+n], g*128+p]."""
    xs = np.zeros((NBLK * P, 256), np.float32)
    xs[:NPC, :FIN] = np.asarray(x, np.float32)[perm_k]
    a = xs.reshape(NBLK, P, 2, P).transpose(3, 0, 2, 1)   # [p, b, g, n]
    return np.ascontiguousarray(a.reshape(P, NBLK * 256)).astype(np.float16)


def _build(plan):
    tiles_bs = plan["tiles_bs"]; ntiles = plan["ntiles"]
    calls = plan["calls"]; nchunk = plan["nchunk"]

    nc = bacc.Bacc("TRN2", target_bir_lowering=False, debug=False,
                   enable_asserts=False, num_devices=NCORES, num_swdge_queues=4,
                   dynamic_dma_scratch_size=32768)

    xt_in = nc.dram_tensor("xt", [P, NBLK * 256], f16, kind="ExternalInput")
    w1_in = nc.dram_tensor("w1cat", [FIN, 264], f16, kind="ExternalInput")
    w2_in = nc.dram_tensor("w2cat", [H1 * HID, 66], f16, kind="ExternalInput")
    b1_in = nc.dram_tensor("b1", [1, H1 * HID], f32, kind="ExternalInput")
    b2_in = nc.dram_tensor("b2", [1, HID], f32, kind="ExternalInput")
    wc_in = nc.dram_tensor("wc", [1, HID * NC_OUT], f32, kind="ExternalInput")
    bc_in = nc.dram_tensor("bc", [1, NC_OUT], f32, kind="ExternalInput")
    idx_in = nc.dram_tensor("idxs", [P, ntiles * 8], i16, kind="ExternalInput")
    dd_in = nc.dram_tensor("dds", [P, ntiles * 256], f16, kind="ExternalInput")
    out_t = nc.dram_tensor("out", [NPC, NC_OUT], f32, kind="ExternalOutput")

    RG = [list(range(NCORES))]

    with tile.TileContext(nc) as tc:
        with tc.tile_pool(name="const", bufs=1) as cp, \
             tc.tile_pool(name="work", bufs=3) as wp, \
             tc.tile_pool(name="gst", bufs=2) as gp, \
             tc.tile_pool(name="dts", bufs=2) as dp, \
             tc.tile_pool(name="dram", bufs=1, space="DRAM") as dr, \
             tc.tile_pool(name="psA", bufs=4, space="PSUM") as psA, \
             tc.tile_pool(name="psB", bufs=1, space="PSUM") as psB, \
             tc.tile_pool(name="psC", bufs=2, space="PSUM") as psC, \
             tc.tile_pool(name="psD", bufs=1, space="PSUM") as psD:

            nc.gpsimd.load_library(library_config.mlp)

            # per-sub local tables (AG inputs) and gathered tables (Shared)
            loc1 = [dr.tile([SUBLEN[s], RU1], f16, name=f"hs1loc{s}")
                    for s in range(NSUB)]
            ful1 = [dr.tile([SUBLEN[s] * NCORES, RU1], f16,
                            name=f"hs1ful{s}")
                    for s in range(NSUB)]
            loc2 = [dr.tile([SUBLEN[s], RU2], f16, name=f"hs2loc{s}")
                    for s in range(NSUB)]
            ful2 = [dr.tile([SUBLEN[s] * NCORES, RU2], f16,
                            name=f"hs2ful{s}")
                    for s in range(NSUB)]

            # ---------- constants
            from concourse.masks import make_identity
            ident = cp.tile([P, P], f32)
            make_identity(nc, ident[:])
            ident16 = cp.tile([P, P], f16)
            nc.vector.tensor_copy(out=ident16[:], in_=ident[:])
            w1c = cp.tile([P, 264], f16)
            w1c2 = cp.tile([FIN - P, 264], f16)
            nc.sync.dma_start(out=w1c[:], in_=w1_in[0:P, :])
            nc.sync.dma_start(out=w1c2[:], in_=w1_in[P:FIN, :])
            w2c = cp.tile([P, 66], f16)
            w2c2 = cp.tile([P, 66], f16)
            nc.sync.dma_start(out=w2c[:], in_=w2_in[0:P, :])
            nc.sync.dma_start(out=w2c2[:], in_=w2_in[P:2 * P, :])
            onecol = cp.tile([1, P], f16)
            nc.vector.memset(onecol[:], 1.0)
            # per-partition constants for broadcast tensor_tensor ops
            # (tensor_scalar is ~6x slower on DVE than broadcast TT)
            czero = cp.tile([P, 1], f32)
            nc.vector.memset(czero[:], 0.0)
            cone = cp.tile([P, 1], f32)
            nc.vector.memset(cone[:], 1.0)
            cslope = cp.tile([P, 1], f32)
            nc.vector.memset(cslope[:], NEG_SLOPE)

            def replicate(dram_ap, ncols, tag):
                srcf = wp.tile([1, 256], f16, tag="repf16")
                srci = wp.tile([1, 256], f32, tag="repf32")
                nc.sync.dma_start(out=srci[:, :ncols], in_=dram_ap)
                nc.vector.tensor_copy(out=srcf[:, :ncols], in_=srci[:, :ncols])
                ps = psD.tile([P, 512], f32, tag="scr")
                nc.tensor.matmul(out=ps[:, :ncols], lhsT=onecol[:], rhs=srcf[:, :ncols],
                                 start=True, stop=True)
                dst = cp.tile([P, ncols], f32, tag=tag)
                nc.vector.tensor_copy(out=dst[:], in_=ps[:, :ncols])
                return dst

            b1rep = replicate(b1_in[:], H1 * HID, "b1rep")
            b2rep = replicate(b2_in[:], HID, "b2rep")
            wcrep = replicate(wc_in[:], HID * NC_OUT, "wcrep")
            bcrep = replicate(bc_in[:], NC_OUT, "bcrep")

            dstash1 = cp.tile([P, NBLK * H1], f16)    # per-block d (layer 1)
            dstash2 = cp.tile([P, NBLK], f16)         # per-block d (layer 2)
            logits = cp.tile([P, NBLK * NC_OUT], f32)
            outsb = cp.tile([P, NBLK * NC_OUT], f32)

            def sub_of_block(b):
                return min(b // 25, 3)

            # ================= pass 0 =================
            for b in range(NBLK):
                s = sub_of_block(b)
                r0 = b * P - (0 if s == 0 else [0, 3200, 6400, 9600][s])
                rows = min(P, NPC - b * P)
                xT = wp.tile([P, 256], f16, tag="xT")
                nc.sync.dma_start(out=xT[:], in_=xt_in[:, b * 256:(b + 1) * 256])
                acc = psA.tile([P, 264], f32, tag="agg")
                nc.tensor.matmul(out=acc[:], lhsT=xT[:, 0:P], rhs=w1c[:],
                                 start=True, stop=False)
                nc.tensor.matmul(out=acc[:], lhsT=xT[0:FIN - P, P:2 * P],
                                 rhs=w1c2[:], start=False, stop=True)
                row = wp.tile([P, RU1], f16, tag="row1")
                nc.vector.tensor_tensor(out=row[:, 0:256], in0=acc[:, 0:256],
                                        in1=b1rep[:], op=ALU.add)
                nc.vector.tensor_copy(out=row[:, 256:264], in_=acc[:, 256:264])
                nc.vector.memset(row[:, 264:RU1], 0.0)
                nc.vector.tensor_copy(out=dstash1[:, b * H1:(b + 1) * H1],
                                      in_=acc[:, 260:264])
                nc.sync.dma_start(out=loc1[s][r0:r0 + rows, :], in_=row[0:rows, :])
                if b in (24, 49, 74, 97):
                    nc.gpsimd.collective_compute(
                        "AllGather", ALU.bypass, replica_groups=RG,
                        ins=[loc1[s][:]], outs=[ful1[s][:]])

            # ================= edge loops =================
            Tcmax = max(T for (_, _, _, T) in calls)

            def edge_layer(layer):
                RU = RU1 if layer == 1 else RU2
                NH = H1 if layer == 1 else 1
                MW = 260 if layer == 1 else 65
                HC = 256 if layer == 1 else 64
                SOF = 256 if layer == 1 else 64
                tabs = ful1 if layer == 1 else ful2
                dstash = dstash1 if layer == 1 else dstash2
                for c in range(nchunk):
                    blks = list(range(c * CHUNK, min((c + 1) * CHUNK, NBLK)))
                    # chunk-wide per-block agg tiles + counters
                    aggs = {}
                    first = {b: True for b in blks}
                    left = {b: int(tiles_bs[b].sum()) for b in blks}
                    for b in blks:
                        aggs[b] = psA.tile([P, 264], f32, tag="agg",
                                           name=f"agg_{layer}_{b}")
                    for s in range(NSUB):
                        ci = c * NSUB + s
                        (_, _, ct0, T_call) = calls[ci]
                        assert calls[ci][0] == c and calls[ci][1] == s
                        if T_call == 0:
                            continue
                        idxs = dp.tile([P, Tcmax * 8], i16, tag=f"ix{layer}")
                        nc.sync.dma_start(
                            out=idxs[:, 0:T_call * 8],
                            in_=idx_in[:, ct0 * 8:(ct0 + T_call) * 8])
                        G = gp.tile([P, Tcmax, RU], f16, tag=f"G{layer}")
                        # per-block gather calls: keep each call's descriptor
                        # count under the SWDGE carveout (1024 descriptors)
                        goff = 0
                        for b in blks:
                            Tb = int(tiles_bs[b, s])
                            nc.gpsimd.dma_gather(
                                G[:, goff:goff + Tb, :], tabs[s][:],
                                idxs[:, goff * 8:(goff + Tb) * 8],
                                Tb * P, Tb * P, RU,
                                queue_num=(ci + goff) % 4)
                            goff += Tb
                        assert goff == T_call
                        ddb = dp.tile([P, Tcmax, 256], f16, tag=f"dd{layer}")
                        nc.sync.dma_start(
                            out=ddb[:, 0:T_call, :],
                            in_=dd_in[:, ct0 * 256:(ct0 + T_call) * 256]
                                .rearrange("p (t c) -> p t c", c=256))
                        tt = 0
                        for b in blks:
                            T = int(tiles_bs[b, s])
                            dblk = dstash[:, b * NH:(b + 1) * NH]
                            dxp = psC.tile([P, GM * H1], f32, tag="dx")
                            for ti in range(T):
                                nc.tensor.matmul(
                                    out=dxp[:, ti * NH:(ti + 1) * NH],
                                    lhsT=ddb[:, tt + ti, P:2 * P], rhs=dblk,
                                    start=True, stop=True, skip_group_check=True)
                            ee = wp.tile([P, GM * H1], f32, tag=f"e{layer}")
                            nc.vector.tensor_tensor(
                                out=ee[:, :T * NH].rearrange("p (t h) -> p t h", h=NH),
                                in0=G[:, tt:tt + T, SOF:SOF + NH],
                                in1=dxp[:, :T * NH].rearrange("p (t h) -> p t h", h=NH),
                                op=ALU.add)
                            # lrelu(x) = max(x, 0.2*x), via broadcast TTs
                            t1 = wp.tile([P, GM * H1], f32, tag=f"t{layer}")
                            nc.vector.tensor_tensor(
                                out=t1[:, :T * NH], in0=ee[:, :T * NH],
                                in1=cslope[:].to_broadcast([P, T * NH]),
                                op=ALU.mult)
                            lr = wp.tile([P, GM * H1], f32, tag=f"l{layer}")
                            nc.vector.tensor_tensor(out=lr[:, :T * NH],
                                                    in0=ee[:, :T * NH],
                                                    in1=t1[:, :T * NH], op=ALU.max)
                            pp = wp.tile([P, GM * H1], f32, tag=f"p{layer}")
                            nc.scalar.activation(out=pp[:, :T * NH],
                                                 in_=lr[:, :T * NH], func=AF.Exp)
                            M = wp.tile([P, GM, MW + H1], f16, tag=f"M{layer}")
                            # p lands in M's denominator columns via a second
                            # Exp on the (otherwise idle) scalar engine
                            nc.scalar.activation(
                                out=M[:, 0:T, HC:HC + NH],
                                in_=lr[:, :T * NH].rearrange("p (t h) -> p t h", h=NH),
                                func=AF.Exp)
                            nc.vector.tensor_tensor(
                                out=M[:, 0:T, 0:HC].rearrange(
                                    "p t (h c) -> p t h c", c=HID),
                                in0=G[:, tt:tt + T, 0:HC].rearrange(
                                    "p t (h c) -> p t h c", c=HID),
                                in1=pp[:, :T * NH].rearrange("p (t h) -> p t h", h=NH)
                                    .unsqueeze(3).to_broadcast([P, T, NH, HID]),
                                op=ALU.mult)
                            for ti in range(T):
                                nc.tensor.matmul(
                                    out=aggs[b][:, 0:MW + NH],
                                    lhsT=ddb[:, tt + ti, 0:P],
                                    rhs=M[:, ti, 0:MW + NH],
                                    start=first[b], stop=(left[b] == 1),
                                    skip_group_check=True)
                                first[b] = False
                                left[b] -= 1
                            tt += T
                        assert tt == T_call
                    # epilogues for this chunk
                    for b in blks:
                        rows = min(P, NPC - b * P)
                        if layer == 1:
                            _epilogue1(b, rows, aggs[b])
                        else:
                            _epilogue2(b, rows, aggs[b])

            # ---------- epilogues
            def _elu16(dst, src_ap, ncols, tagp):
                mn = wp.tile([P, ncols], f16, tag=f"{tagp}mn")
                nc.vector.tensor_tensor(out=mn[:], in0=src_ap,
                                        in1=czero[:].to_broadcast([P, ncols]),
                                        op=ALU.min)
                ex = wp.tile([P, ncols], f16, tag=f"{tagp}ex")
                nc.scalar.activation(out=ex[:], in_=mn[:], func=AF.Exp)
                ex1 = wp.tile([P, ncols], f16, tag=f"{tagp}e1")
                nc.vector.tensor_tensor(out=ex1[:], in0=ex[:],
                                        in1=cone[:].to_broadcast([P, ncols]),
                                        op=ALU.subtract)
                nc.vector.tensor_tensor(out=dst, in0=src_ap, in1=ex1[:], op=ALU.max)

            def _epilogue1(b, rows, aggp):
                s = sub_of_block(b)
                r0 = b * P - [0, 3200, 6400, 9600][s]
                rec = wp.tile([P, H1], f32, tag="rec1")
                nc.vector.reciprocal(out=rec[:], in_=aggp[:, 256:260])
                h2a = wp.tile([P, H1 * HID], f16, tag="h2a")
                nc.vector.tensor_tensor(
                    out=h2a[:].rearrange("p (h c) -> p h c", c=HID),
                    in0=aggp[:, 0:256].rearrange("p (h c) -> p h c", c=HID),
                    in1=rec[:].unsqueeze(2).to_broadcast([P, H1, HID]),
                    op=ALU.mult)
                h2 = wp.tile([P, H1 * HID], f16, tag="h2")
                _elu16(h2[:], h2a[:], H1 * HID, "e1")
                mm2 = psB.tile([P, 66], f32, tag="agg2")
                for g in range(2):
                    pt = psD.tile([P, 512], f32, tag="scr")
                    ptv = pt[:, 0:P // 2].bitcast(f16)
                    nc.tensor.transpose(out=ptv, in_=h2[:, g * P:(g + 1) * P],
                                        identity=ident16[:])
                    h2T = wp.tile([P, P], f16, tag=f"h2T{g}")
                    nc.vector.tensor_copy(out=h2T[:], in_=ptv)
                    nc.tensor.matmul(out=mm2[:], lhsT=h2T[:],
                                     rhs=(w2c if g == 0 else w2c2)[:],
                                     start=(g == 0), stop=(g == 1))
                row2 = wp.tile([P, RU2], f16, tag="row2")
                nc.vector.tensor_tensor(out=row2[:, 0:HID], in0=mm2[:, 0:HID],
                                        in1=b2rep[:], op=ALU.add)
                nc.vector.tensor_copy(out=row2[:, 64:66], in_=mm2[:, 64:66])
                nc.vector.memset(row2[:, 66:RU2], 0.0)
                nc.vector.tensor_copy(out=dstash2[:, b:b + 1], in_=mm2[:, 65:66])
                nc.sync.dma_start(out=loc2[s][r0:r0 + rows, :], in_=row2[0:rows, :])
                if b in (24, 49, 74, 97):
                    nc.gpsimd.collective_compute(
                        "AllGather", ALU.bypass, replica_groups=RG,
                        ins=[loc2[s][:]], outs=[ful2[s][:]])

            def _epilogue2(b, rows, aggp):
                rec = wp.tile([P, 1], f32, tag="rec2")
                nc.vector.reciprocal(out=rec[:], in_=aggp[:, 64:65])
                h3a = wp.tile([P, HID], f32, tag="h3a")
                nc.vector.tensor_tensor(
                    out=h3a[:], in0=aggp[:, 0:HID],
                    in1=rec[:].to_broadcast([P, HID]), op=ALU.mult)
                h3 = wp.tile([P, HID], f32, tag="h3")
                _elu16(h3[:], h3a[:], HID, "e2")
                tmp = wp.tile([P, HID], f32, tag="lgt")
                wcv = wcrep[:].rearrange("p (k c) -> p k c", c=NC_OUT)
                lg = wp.tile([P, NC_OUT], f32, tag="lg")
                for j in range(NC_OUT):
                    nc.vector.tensor_tensor(out=tmp[:], in0=h3[:],
                                            in1=wcv[:, :, j], op=ALU.mult)
                    nc.vector.tensor_reduce(out=lg[:, j:j + 1], in_=tmp[:],
                                            op=ALU.add, axis=AXX)
                nc.vector.tensor_tensor(
                    out=logits[:, b * NC_OUT:(b + 1) * NC_OUT],
                    in0=lg[:], in1=bcrep[:], op=ALU.add)

            edge_layer(1)
            edge_layer(2)

            # ---------- batched log-softmax over all blocks
            lv = logits[:].rearrange("p (b c) -> p b c", c=NC_OUT)
            ov = outsb[:].rearrange("p (b c) -> p b c", c=NC_OUT)
            mx = wp.tile([P, NBLK], f32, tag="fmx")
            mxv = mx[:].rearrange("p (b o) -> p b o", o=1)
            nc.vector.tensor_tensor(out=mxv, in0=lv[:, :, 0:1], in1=lv[:, :, 1:2],
                                    op=ALU.max)
            am = wp.tile([P, NBLK * NC_OUT], f32, tag="fam")
            amv = am[:].rearrange("p (b c) -> p b c", c=NC_OUT)
            nc.vector.tensor_tensor(out=amv, in0=lv,
                                    in1=mxv.to_broadcast([P, NBLK, NC_OUT]),
                                    op=ALU.subtract)
            ex = wp.tile([P, NBLK * NC_OUT], f32, tag="fex")
            nc.scalar.activation(out=ex[:], in_=am[:], func=AF.Exp)
            exv = ex[:].rearrange("p (b c) -> p b c", c=NC_OUT)
            sm = wp.tile([P, NBLK], f32, tag="fsm")
            smv = sm[:].rearrange("p (b o) -> p b o", o=1)
            nc.vector.tensor_tensor(out=smv, in0=exv[:, :, 0:1], in1=exv[:, :, 1:2],
                                    op=ALU.add)
            lsm = wp.tile([P, NBLK], f32, tag="flsm")
            nc.scalar.activation(out=lsm[:], in_=sm[:], func=AF.Ln)
            nc.vector.tensor_tensor(
                out=ov, in0=amv,
                in1=lsm[:].rearrange("p (b o) -> p b o", o=1)
                    .to_broadcast([P, NBLK, NC_OUT]),
                op=ALU.subtract)

            nc.sync.dma_start(
                out=bass.AP(out_t[:].tensor, 0,
                            [[NC_OUT, P], [P * NC_OUT, NBLK - 1], [1, NC_OUT]]),
                in_=outsb[:, 0:(NBLK - 1) * NC_OUT])
            lastrows = NPC - (NBLK - 1) * P
            nc.sync.dma_start(
                out=bass.AP(out_t[:].tensor, (NBLK - 1) * P * NC_OUT,
                            [[NC_OUT, lastrows], [1, NC_OUT]]),
                in_=outsb[0:lastrows, (NBLK - 1) * NC_OUT:NBLK * NC_OUT])

    nc.compile()
    return nc


def kernel(**inputs):
    x = np.asarray(inputs["x"], np.float32)
    edge_index = np.asarray(inputs["edge_index"])
    plan, data, perm = _host_prep(edge_index)
    Wcat1, Wcat2 = _host_weights(
        inputs["W1"], inputs["a_src1"], inputs["a_dst1"],
        inputs["W2"], inputs["a_src2"], inputs["a_dst2"])
    b1 = np.asarray(inputs["b1"], np.float32).reshape(1, -1)
    b2 = np.asarray(inputs["b2"], np.float32).reshape(1, -1)
    wc = np.asarray(inputs["Wc"], np.float32).reshape(1, -1).copy()
    bc = np.asarray(inputs["bc"], np.float32).reshape(1, -1)

    nc = _build(plan)
    in_maps = []
    for k in range(NCORES):
        in_maps.append({
            "xt": _host_xt(x, perm[k]),
            "w1cat": Wcat1, "w2cat": Wcat2,
            "b1": b1, "b2": b2, "wc": wc, "bc": bc,
            "idxs": data[k]["idx_stream"],
            "dds": data[k]["ddT"],
        })
    res = bass_utils.run_bass_kernel_spmd(
        nc, in_maps, core_ids=list(range(NCORES)),
        trace=globals().get("TRACE", False))
    globals()["LAST_RES"] = res
    outp = np.concatenate([np.asarray(r["out"], np.float32) for r in res.results],
                          axis=0)
    out = np.empty((N_NODES, NC_OUT), np.float32)
    out[perm.reshape(-1)] = outp
    return out


if __name__ == "__main__":
    pass


# revision 15
# speedup vs baseline: 1.0622x; 1.0622x over previous
"""2-layer GAT (PyG semantics) on 8 Trainium2 NeuronCores via Bass/Tile.

v2 design:
  - dst-sharded edges with degree-balanced node->(core,block) assignment
    (host permutation, inverse-applied to the output).
  - pass0: h1|s1|d1 = x @ Wcat1 from host-pretransposed f16 x; rows are
    h-major 768B: [256 h f16 | 4 s f16 | 4 d f16 | pad]. d stashed in SBUF.
  - AllGather split into 4 row-chunks per layer (Shared outputs), issued as
    soon as the producing blocks are done, so transfers overlap compute.
  - edge pass per chunk-of-4-blocks: ONE dma_gather per (chunk, src-subrange)
    (amortizes the ~1us fixed SWDGE cost), one-hot D/DT streamed from DRAM in
    a host-pretransposed layout (single contiguous DMA per chunk).
  - attention: dxp = D @ dblk on PE; e = s+dxp; lrelu via fused
    tensor_scalar(min,*-0.8)+add on DVE (scalar engine runs Exp ONLY -> no
    activation-table thrash); p=exp(e); M = [G*p | p]; agg += DT^T @ M
    accumulated in PSUM across all 4 src-subranges of a block.
  - log-softmax batched once over all blocks at the end (single Ln).
"""
import sys
sys.path.insert(0, "/opt/trn_rl_repo")
import numpy as np

import concourse.bass as bass
import concourse.bacc as bacc
import concourse.tile as tile
from concourse import mybir, library_config
from concourse import bass_utils

P = 128
NEG_SLOPE = 0.2
N_NODES = 100000
FIN = 165
HID = 64
H1 = 4
NC_OUT = 2
NCORES = 8
NPC = N_NODES // NCORES          # 12500
NBLK = (NPC + P - 1) // P        # 98
NSUB = 4                         # AllGather row-chunks / gather tables
SUBLEN = [3200, 3200, 3200, 2900]
SUBB = [25, 25, 25, 23]          # blocks per sub
RU1 = 384                        # f16 units per L1 row (768B)
RU2 = 128                        # f16 units per L2 row (256B)
CHUNK = 4                        # blocks per gather call
GM = 8                           # max tiles per (block, sub) matmul group

f16 = mybir.dt.float16
f32 = mybir.dt.float32
i16 = mybir.dt.int16
AF = mybir.ActivationFunctionType
ALU = mybir.AluOpType
AXX = mybir.AxisListType.X


def _host_assign(edge_index):
    """Degree-balanced node -> (core, block, slot) assignment."""
    src = np.concatenate([np.asarray(edge_index[0]), np.arange(N_NODES)]).astype(np.int64)
    dst = np.concatenate([np.asarray(edge_index[1]), np.arange(N_NODES)]).astype(np.int64)
    deg = np.bincount(dst, minlength=N_NODES)
    order = np.argsort(-deg, kind="stable")
    # groups of 8 consecutive (similar-degree) nodes -> one per core; within a
    # core, deal degree ranks round-robin across blocks so every block has the
    # same degree profile (last block has only 84 slots).
    ranked = order.reshape(NPC, NCORES).T            # [NCORES, NPC] by rank
    r = np.arange(NPC)
    full = 84 * NBLK                                 # rounds covering all blocks
    blk = np.where(r < full, r % NBLK, (r - full) % (NBLK - 1))
    slot = np.where(r < full, r // NBLK, 84 + (r - full) // (NBLK - 1))
    lpos_of_rank = blk * P + slot
    perm = np.empty((NCORES, NPC), np.int64)
    for k in range(NCORES):
        perm[k][lpos_of_rank] = ranked[k]
    # node -> (core, localpos)
    core_of = np.empty(N_NODES, np.int64)
    lpos_of = np.empty(N_NODES, np.int64)
    for k in range(NCORES):
        core_of[perm[k]] = k
        lpos_of[perm[k]] = np.arange(NPC)
    return src, dst, perm, core_of, lpos_of


def _host_prep(edge_index):
    src, dst, perm, core_of, lpos_of = _host_assign(edge_index)
    sub_starts = np.array([0, 3200, 6400, 9600, 12500])
    # table position of a (global) node, per-sub tables of [8*len_s] rows
    l_all = lpos_of
    s_all = np.minimum(l_all // 3200, 3)
    len_all = np.array(SUBLEN)[s_all]
    pos_all = core_of * len_all + (l_all - sub_starts[s_all])

    dcore = core_of[dst]
    dl = lpos_of[dst]
    blk_all = dl // P
    slot_all = dl % P

    # per (core, block, sub) counts
    cnt = np.zeros((NCORES, NBLK, NSUB), np.int64)
    percore = []
    for k in range(NCORES):
        sel = np.nonzero(dcore == k)[0]
        e_pos = pos_all[src[sel]]
        e_sub = s_all[src[sel]]
        e_blk = blk_all[sel]
        e_slot = slot_all[sel]
        o = np.lexsort((e_pos, e_sub, e_blk))
        percore.append((e_pos[o], e_sub[o], e_blk[o], e_slot[o]))
        np.add.at(cnt[k], (e_blk, e_sub), 1)
    reg = np.maximum(cnt.max(axis=0), 1)
    tiles_bs = (reg + P - 1) // P                     # [NBLK, NSUB]
    assert tiles_bs.max() <= GM, tiles_bs.max()

    # chunk structure: blocks [4c, 4c+4)
    nchunk = (NBLK + CHUNK - 1) // CHUNK
    # tile order: for c: for s: for b in chunk
    t_of = {}
    t0 = 0
    calls = []                                        # (c, s, t0, T)
    for c in range(nchunk):
        bs = range(c * CHUNK, min((c + 1) * CHUNK, NBLK))
        for s in range(NSUB):
            call_t0 = t0
            for b in bs:
                t_of[(b, s)] = t0
                t0 += int(tiles_bs[b, s])
            calls.append((c, s, call_t0, t0 - call_t0))
    ntiles = t0
    plan = dict(tiles_bs=tiles_bs, ntiles=ntiles, calls=calls, nchunk=nchunk)

    data = []
    for k in range(NCORES):
        e_pos, e_sub, e_blk, e_slot = percore[k]
        flat_pos = np.zeros(ntiles * P, np.int64)     # pad pos = 0 (valid row)
        flat_slot = np.full(ntiles * P, -1, np.int64)
        # bucket boundaries per (b, s)
        keys = e_blk * NSUB + e_sub
        bounds = np.searchsorted(keys, np.arange(NBLK * NSUB + 1))
        for b in range(NBLK):
            for s in range(NSUB):
                lo, hi = bounds[b * NSUB + s], bounds[b * NSUB + s + 1]
                L = hi - lo
                if L == 0:
                    continue
                base = t_of[(b, s)] * P
                flat_pos[base:base + L] = e_pos[lo:hi]
                flat_slot[base:base + L] = e_slot[lo:hi]
        # idx stream: wrap 16 per CALL region, then replicate to 128 chans
        idx16 = np.zeros((16, ntiles * P // 16), np.int16)
        for (c, s, ct0, T) in calls:
            if T == 0:
                continue
            n = T * P
            seg = flat_pos[ct0 * P: ct0 * P + n]
            assert seg.max() < 32768
            idx16[:, ct0 * 8:(ct0 + T) * 8] = \
                seg.reshape(n // 16, 16).T.astype(np.int16)
        idx_stream = np.tile(idx16, (8, 1))           # [128, ntiles*8]

        # ddT [128, ntiles*256]: cols t*256+i  = DT[e=part, i]  (i=slot)
        #                        cols t*256+128+e = D[i=part, e]
        ddT = np.zeros((P, ntiles * 256), np.float16)
        t_idx = np.repeat(np.arange(ntiles), P)
        e_idx = np.tile(np.arange(P), ntiles)
        v = flat_slot >= 0
        ddT[e_idx[v], t_idx[v] * 256 + flat_slot[v]] = 1.0
        ddT[flat_slot[v], t_idx[v] * 256 + 128 + e_idx[v]] = 1.0
        data.append(dict(idx_stream=idx_stream, ddT=ddT))
    return plan, data, perm


def _host_weights(W1, a_src1, a_dst1, W2, a_src2, a_dst2):
    W1 = np.asarray(W1, np.float32); W2 = np.asarray(W2, np.float32)
    a_src1 = np.asarray(a_src1, np.float32); a_dst1 = np.asarray(a_dst1, np.float32)
    a_src2 = np.asarray(a_src2, np.float32); a_dst2 = np.asarray(a_dst2, np.float32)
    Wcat1 = np.zeros((FIN, 264), np.float32)
    Wcat1[:, 0:256] = W1
    for h in range(H1):
        Wcat1[:, 256 + h] = W1[:, h * HID:(h + 1) * HID] @ a_src1[h]
        Wcat1[:, 260 + h] = W1[:, h * HID:(h + 1) * HID] @ a_dst1[h]
    Wcat2 = np.zeros((H1 * HID, 66), np.float32)
    Wcat2[:, :HID] = W2
    Wcat2[:, 64] = W2 @ a_src2[0]
    Wcat2[:, 65] = W2 @ a_dst2[0]
    return Wcat1.astype(np.float16), Wcat2.astype(np.float16)


def _host_xt(x, perm_k):
    """[128, NBLK*256] f16: xT[p, b*256+g*128+n] = x[perm[b*128+n], g*128+p]."""
    xs = np.zeros((NBLK * P, 256), np.float32)
    xs[:NPC, :FIN] = np.asarray(x, np.float32)[perm_k]
    a = xs.reshape(NBLK, P, 2, P).transpose(3, 0, 2, 1)   # [p, b, g, n]
    return np.ascontiguousarray(a.reshape(P, NBLK * 256)).astype(np.float16)


def _build(plan):
    tiles_bs = plan["tiles_bs"]; ntiles = plan["ntiles"]
    calls = plan["calls"]; nchunk = plan["nchunk"]

    nc = bacc.Bacc("TRN2", target_bir_lowering=False, debug=False,
                   enable_asserts=False, num_devices=NCORES, num_swdge_queues=4,
                   dynamic_dma_scratch_size=65536)

    xt_in = nc.dram_tensor("xt", [P, NBLK * 256], f16, kind="ExternalInput")
    w1_in = nc.dram_tensor("w1cat", [FIN, 264], f16, kind="ExternalInput")
    w2_in = nc.dram_tensor("w2cat", [H1 * HID, 66], f16, kind="ExternalInput")
    b1_in = nc.dram_tensor("b1", [1, H1 * HID], f32, kind="ExternalInput")
    b2_in = nc.dram_tensor("b2", [1, HID], f32, kind="ExternalInput")
    wc_in = nc.dram_tensor("wc", [1, HID * NC_OUT], f32, kind="ExternalInput")
    bc_in = nc.dram_tensor("bc", [1, NC_OUT], f32, kind="ExternalInput")
    idx_in = nc.dram_tensor("idxs", [P, ntiles * 8], i16, kind="ExternalInput")
    dd_in = nc.dram_tensor("dds", [P, ntiles * 256], f16, kind="ExternalInput")
    out_t = nc.dram_tensor("out", [NPC, NC_OUT], f32, kind="ExternalOutput")

    RG = [list(range(NCORES))]

    with tile.TileContext(nc) as tc:
        with tc.tile_pool(name="const", bufs=1) as cp, \
             tc.tile_pool(name="work", bufs=3) as wp, \
             tc.tile_pool(name="gst", bufs=2) as gp, \
             tc.tile_pool(name="dts", bufs=2) as dp, \
             tc.tile_pool(name="dram", bufs=1, space="DRAM") as dr, \
             tc.tile_pool(name="psA", bufs=4, space="PSUM") as psA, \
             tc.tile_pool(name="psB", bufs=1, space="PSUM") as psB, \
             tc.tile_pool(name="psC", bufs=2, space="PSUM") as psC, \
             tc.tile_pool(name="psD", bufs=1, space="PSUM") as psD:

            nc.gpsimd.load_library(library_config.mlp)

            # per-sub local tables (AG inputs) and gathered tables (Shared)
            loc1 = [dr.tile([SUBLEN[s], RU1], f16, name=f"hs1loc{s}")
                    for s in range(NSUB)]
            ful1 = [dr.tile([SUBLEN[s] * NCORES, RU1], f16,
                            name=f"hs1ful{s}")
                    for s in range(NSUB)]
            loc2 = [dr.tile([SUBLEN[s], RU2], f16, name=f"hs2loc{s}")
                    for s in range(NSUB)]
            ful2 = [dr.tile([SUBLEN[s] * NCORES, RU2], f16,
                            name=f"hs2ful{s}")
                    for s in range(NSUB)]

            # ---------- constants
            from concourse.masks import make_identity
            ident = cp.tile([P, P], f32)
            make_identity(nc, ident[:])
            ident16 = cp.tile([P, P], f16)
            nc.vector.tensor_copy(out=ident16[:], in_=ident[:])
            w1c = cp.tile([P, 264], f16)
            w1c2 = cp.tile([FIN - P, 264], f16)
            nc.sync.dma_start(out=w1c[:], in_=w1_in[0:P, :])
            nc.sync.dma_start(out=w1c2[:], in_=w1_in[P:FIN, :])
            w2c = cp.tile([P, 66], f16)
            w2c2 = cp.tile([P, 66], f16)
            nc.sync.dma_start(out=w2c[:], in_=w2_in[0:P, :])
            nc.sync.dma_start(out=w2c2[:], in_=w2_in[P:2 * P, :])
            onecol = cp.tile([1, P], f16)
            nc.vector.memset(onecol[:], 1.0)
            # per-partition constants for broadcast tensor_tensor ops
            # (tensor_scalar is ~6x slower on DVE than broadcast TT)
            czero = cp.tile([P, 1], f32)
            nc.vector.memset(czero[:], 0.0)
            cone = cp.tile([P, 1], f32)
            nc.vector.memset(cone[:], 1.0)
            cslope = cp.tile([P, 1], f32)
            nc.vector.memset(cslope[:], NEG_SLOPE)

            def replicate(dram_ap, ncols, tag):
                srcf = wp.tile([1, 256], f16, tag="repf16")
                srci = wp.tile([1, 256], f32, tag="repf32")
                nc.sync.dma_start(out=srci[:, :ncols], in_=dram_ap)
                nc.vector.tensor_copy(out=srcf[:, :ncols], in_=srci[:, :ncols])
                ps = psD.tile([P, 512], f32, tag="scr")
                nc.tensor.matmul(out=ps[:, :ncols], lhsT=onecol[:], rhs=srcf[:, :ncols],
                                 start=True, stop=True)
                dst = cp.tile([P, ncols], f32, tag=tag)
                nc.vector.tensor_copy(out=dst[:], in_=ps[:, :ncols])
                return dst

            b1rep = replicate(b1_in[:], H1 * HID, "b1rep")
            b2rep = replicate(b2_in[:], HID, "b2rep")
            wcrep = replicate(wc_in[:], HID * NC_OUT, "wcrep")
            bcrep = replicate(bc_in[:], NC_OUT, "bcrep")

            dstash1 = cp.tile([P, NBLK * H1], f16)    # per-block d (layer 1)
            dstash2 = cp.tile([P, NBLK], f16)         # per-block d (layer 2)
            logits = cp.tile([P, NBLK * NC_OUT], f32)
            outsb = cp.tile([P, NBLK * NC_OUT], f32)

            def sub_of_block(b):
                return min(b // 25, 3)

            # ================= pass 0 =================
            for b in range(NBLK):
                s = sub_of_block(b)
                r0 = b * P - (0 if s == 0 else [0, 3200, 6400, 9600][s])
                rows = min(P, NPC - b * P)
                xT = wp.tile([P, 256], f16, tag="xT")
                nc.sync.dma_start(out=xT[:], in_=xt_in[:, b * 256:(b + 1) * 256])
                acc = psA.tile([P, 264], f32, tag="agg")
                nc.tensor.matmul(out=acc[:], lhsT=xT[:, 0:P], rhs=w1c[:],
                                 start=True, stop=False)
                nc.tensor.matmul(out=acc[:], lhsT=xT[0:FIN - P, P:2 * P],
                                 rhs=w1c2[:], start=False, stop=True)
                row = wp.tile([P, RU1], f16, tag="row1")
                nc.vector.tensor_tensor(out=row[:, 0:256], in0=acc[:, 0:256],
                                        in1=b1rep[:], op=ALU.add)
                nc.vector.tensor_copy(out=row[:, 256:264], in_=acc[:, 256:264])
                nc.vector.memset(row[:, 264:RU1], 0.0)
                nc.vector.tensor_copy(out=dstash1[:, b * H1:(b + 1) * H1],
                                      in_=acc[:, 260:264])
                nc.sync.dma_start(out=loc1[s][r0:r0 + rows, :], in_=row[0:rows, :])
                if b in (24, 49, 74, 97):
                    nc.gpsimd.collective_compute(
                        "AllGather", ALU.bypass, replica_groups=RG,
                        ins=[loc1[s][:]], outs=[ful1[s][:]])

            # ================= edge loops =================
            Tcmax = max(T for (_, _, _, T) in calls)

            gq = [0]

            def edge_layer(layer):
                RU = RU1 if layer == 1 else RU2
                NH = H1 if layer == 1 else 1
                MW = 260 if layer == 1 else 65
                HC = 256 if layer == 1 else 64
                SOF = 256 if layer == 1 else 64
                tabs = ful1 if layer == 1 else ful2
                dstash = dstash1 if layer == 1 else dstash2
                for c in range(nchunk):
                    blks = list(range(c * CHUNK, min((c + 1) * CHUNK, NBLK)))
                    # chunk-wide per-block agg tiles + counters
                    aggs = {}
                    first = {b: True for b in blks}
                    left = {b: int(tiles_bs[b].sum()) for b in blks}
                    for b in blks:
                        aggs[b] = psA.tile([P, 264], f32, tag="agg",
                                           name=f"agg_{layer}_{b}")
                    for s in range(NSUB):
                        ci = c * NSUB + s
                        (_, _, ct0, T_call) = calls[ci]
                        assert calls[ci][0] == c and calls[ci][1] == s
                        if T_call == 0:
                            continue
                        idxs = dp.tile([P, Tcmax * 8], i16, tag="ix")
                        nc.sync.dma_start(
                            out=idxs[:, 0:T_call * 8],
                            in_=idx_in[:, ct0 * 8:(ct0 + T_call) * 8])
                        G = gp.tile([P, Tcmax, RU], f16, tag=f"G{layer}")
                        # per-block gather calls: keep each call's descriptor
                        # count well under the SWDGE ring; strict round-robin
                        # across the 4 queues to avoid head-of-line blocking
                        goff = 0
                        for b in blks:
                            Tb = int(tiles_bs[b, s])
                            nc.gpsimd.dma_gather(
                                G[:, goff:goff + Tb, :], tabs[s][:],
                                idxs[:, goff * 8:(goff + Tb) * 8],
                                Tb * P, Tb * P, RU,
                                queue_num=gq[0] % 4)
                            gq[0] += 1
                            goff += Tb
                        assert goff == T_call
                        ddb = dp.tile([P, Tcmax, 256], f16, tag="dd")
                        nc.sync.dma_start(
                            out=ddb[:, 0:T_call, :],
                            in_=dd_in[:, ct0 * 256:(ct0 + T_call) * 256]
                                .rearrange("p (t c) -> p t c", c=256))
                        tt = 0
                        for b in blks:
                            T = int(tiles_bs[b, s])
                            dblk = dstash[:, b * NH:(b + 1) * NH]
                            dxp = psC.tile([P, GM * H1], f32, tag="dx")
                            for ti in range(T):
                                nc.tensor.matmul(
                                    out=dxp[:, ti * NH:(ti + 1) * NH],
                                    lhsT=ddb[:, tt + ti, P:2 * P], rhs=dblk,
                                    start=True, stop=True, skip_group_check=True)
                            ee = wp.tile([P, GM * H1], f32, tag=f"e{layer}")
                            nc.vector.tensor_tensor(
                                out=ee[:, :T * NH].rearrange("p (t h) -> p t h", h=NH),
                                in0=G[:, tt:tt + T, SOF:SOF + NH],
                                in1=dxp[:, :T * NH].rearrange("p (t h) -> p t h", h=NH),
                                op=ALU.add)
                            # lrelu(x) = max(x, 0.2*x), via broadcast TTs
                            t1 = wp.tile([P, GM * H1], f32, tag=f"t{layer}")
                            nc.vector.tensor_tensor(
                                out=t1[:, :T * NH], in0=ee[:, :T * NH],
                                in1=cslope[:].to_broadcast([P, T * NH]),
                                op=ALU.mult)
                            lr = wp.tile([P, GM * H1], f32, tag=f"l{layer}")
                            nc.vector.tensor_tensor(out=lr[:, :T * NH],
                                                    in0=ee[:, :T * NH],
                                                    in1=t1[:, :T * NH], op=ALU.max)
                            pp = wp.tile([P, GM * H1], f32, tag=f"p{layer}")
                            nc.scalar.activation(out=pp[:, :T * NH],
                                                 in_=lr[:, :T * NH], func=AF.Exp)
                            M = wp.tile([P, GM, MW + H1], f16, tag=f"M{layer}")
                            # p lands in M's denominator columns via a second
                            # Exp on the (otherwise idle) scalar engine
                            nc.scalar.activation(
                                out=M[:, 0:T, HC:HC + NH],
                                in_=lr[:, :T * NH].rearrange("p (t h) -> p t h", h=NH),
                                func=AF.Exp)
                            nc.vector.tensor_tensor(
                                out=M[:, 0:T, 0:HC].rearrange(
                                    "p t (h c) -> p t h c", c=HID),
                                in0=G[:, tt:tt + T, 0:HC].rearrange(
                                    "p t (h c) -> p t h c", c=HID),
                                in1=pp[:, :T * NH].rearrange("p (t h) -> p t h", h=NH)
                                    .unsqueeze(3).to_broadcast([P, T, NH, HID]),
                                op=ALU.mult)
                            for ti in range(T):
                                nc.tensor.matmul(
                                    out=aggs[b][:, 0:MW + NH],
                                    lhsT=ddb[:, tt + ti, 0:P],
                                    rhs=M[:, ti, 0:MW + NH],
                                    start=first[b], stop=(left[b] == 1),
                                    skip_group_check=True)
                                first[b] = False
                                left[b] -= 1
                            tt += T
                        assert tt == T_call
                    # epilogues for this chunk
                    for b in blks:
                        rows = min(P, NPC - b * P)
                        if layer == 1:
                            _epilogue1(b, rows, aggs[b])
                        else:
                            _epilogue2(b, rows, aggs[b])

            # ---------- epilogues
            def _elu16(dst, src_ap, ncols, tagp):
                mn = wp.tile([P, ncols], f16, tag=f"{tagp}mn")
                nc.vector.tensor_tensor(out=mn[:], in0=src_ap,
                                        in1=czero[:].to_broadcast([P, ncols]),
                                        op=ALU.min)
                ex = wp.tile([P, ncols], f16, tag=f"{tagp}ex")
                nc.scalar.activation(out=ex[:], in_=mn[:], func=AF.Exp)
                ex1 = wp.tile([P, ncols], f16, tag=f"{tagp}e1")
                nc.vector.tensor_tensor(out=ex1[:], in0=ex[:],
                                        in1=cone[:].to_broadcast([P, ncols]),
                                        op=ALU.subtract)
                nc.vector.tensor_tensor(out=dst, in0=src_ap, in1=ex1[:], op=ALU.max)

            def _epilogue1(b, rows, aggp):
                s = sub_of_block(b)
                r0 = b * P - [0, 3200, 6400, 9600][s]
                rec = wp.tile([P, H1], f32, tag="rec1")
                nc.vector.reciprocal(out=rec[:], in_=aggp[:, 256:260])
                h2a = wp.tile([P, H1 * HID], f16, tag="h2a")
                nc.vector.tensor_tensor(
                    out=h2a[:].rearrange("p (h c) -> p h c", c=HID),
                    in0=aggp[:, 0:256].rearrange("p (h c) -> p h c", c=HID),
                    in1=rec[:].unsqueeze(2).to_broadcast([P, H1, HID]),
                    op=ALU.mult)
                h2 = wp.tile([P, H1 * HID], f16, tag="h2")
                _elu16(h2[:], h2a[:], H1 * HID, "e1")
                mm2 = psB.tile([P, 66], f32, tag="agg2")
                for g in range(2):
                    pt = psD.tile([P, 512], f32, tag="scr")
                    ptv = pt[:, 0:P // 2].bitcast(f16)
                    nc.tensor.transpose(out=ptv, in_=h2[:, g * P:(g + 1) * P],
                                        identity=ident16[:])
                    h2T = wp.tile([P, P], f16, tag=f"h2T{g}")
                    nc.vector.tensor_copy(out=h2T[:], in_=ptv)
                    nc.tensor.matmul(out=mm2[:], lhsT=h2T[:],
                                     rhs=(w2c if g == 0 else w2c2)[:],
                                     start=(g == 0), stop=(g == 1))
                row2 = wp.tile([P, RU2], f16, tag="row2")
                nc.vector.tensor_tensor(out=row2[:, 0:HID], in0=mm2[:, 0:HID],
                                        in1=b2rep[:], op=ALU.add)
                nc.vector.tensor_copy(out=row2[:, 64:66], in_=mm2[:, 64:66])
                nc.vector.memset(row2[:, 66:RU2], 0.0)
                nc.vector.tensor_copy(out=dstash2[:, b:b + 1], in_=mm2[:, 65:66])
                nc.sync.dma_start(out=loc2[s][r0:r0 + rows, :], in_=row2[0:rows, :])
                if b in (24, 49, 74, 97):
                    nc.gpsimd.collective_compute(
                        "AllGather", ALU.bypass, replica_groups=RG,
                        ins=[loc2[s][:]], outs=[ful2[s][:]])

            def _epilogue2(b, rows, aggp):
                rec = wp.tile([P, 1], f32, tag="rec2")
                nc.vector.reciprocal(out=rec[:], in_=aggp[:, 64:65])
                h3a = wp.tile([P, HID], f32, tag="h3a")
                nc.vector.tensor_tensor(
                    out=h3a[:], in0=aggp[:, 0:HID],
                    in1=rec[:].to_broadcast([P, HID]), op=ALU.mult)
                h3 = wp.tile([P, HID], f32, tag="h3")
                _elu16(h3[:], h3a[:], HID, "e2")
                tmp = wp.tile([P, HID], f32, tag="lgt")
                wcv = wcrep[:].rearrange("p (k c) -> p k c", c=NC_OUT)
                lg = wp.tile([P, NC_OUT], f32, tag="lg")
                for j in range(NC_OUT):
                    nc.vector.tensor_tensor(out=tmp[:], in0=h3[:],
                                            in1=wcv[:, :, j], op=ALU.mult)
                    nc.vector.tensor_reduce(out=lg[:, j:j + 1], in_=tmp[:],
                                            op=ALU.add, axis=AXX)
                nc.vector.tensor_tensor(
                    out=logits[:, b * NC_OUT:(b + 1) * NC_OUT],
                    in0=lg[:], in1=bcrep[:], op=ALU.add)

            edge_layer(1)
            edge_layer(2)

            # ---------- batched log-softmax over all blocks
            lv = logits[:].rearrange("p (b c) -> p b c", c=NC_OUT)
            ov = outsb[:].rearrange("p (b c) -> p b c", c=NC_OUT)
            mx = wp.tile([P, NBLK], f32, tag="fmx")
            mxv = mx[:].rearrange("p (b o) -> p b o", o=1)
            nc.vector.tensor_tensor(out=mxv, in0=lv[:, :, 0:1], in1=lv[:, :, 1:2],
                                    op=ALU.max)
            am = wp.tile([P, NBLK * NC_OUT], f32, tag="fam")
            amv = am[:].rearrange("p (b c) -> p b c", c=NC_OUT)
            nc.vector.tensor_tensor(out=amv, in0=lv,
                                    in1=mxv.to_broadcast([P, NBLK, NC_OUT]),
                                    op=ALU.subtract)
            ex = wp.tile([P, NBLK * NC_OUT], f32, tag="fex")
            nc.scalar.activation(out=ex[:], in_=am[:], func=AF.Exp)
            exv = ex[:].rearrange("p (b c) -> p b c", c=NC_OUT)
            sm = wp.tile([P, NBLK], f32, tag="fsm")
            smv = sm[:].rearrange("p (b o) -> p b o", o=1)
            nc.vector.tensor_tensor(out=smv, in0=exv[:, :, 0:1], in1=exv[:, :, 1:2],
                                    op=ALU.add)
            lsm = wp.tile([P, NBLK], f32, tag="flsm")
            nc.scalar.activation(out=lsm[:], in_=sm[:], func=AF.Ln)
            nc.vector.tensor_tensor(
                out=ov, in0=amv,
                in1=lsm[:].rearrange("p (b o) -> p b o", o=1)
                    .to_broadcast([P, NBLK, NC_OUT]),
                op=ALU.subtract)

            nc.sync.dma_start(
                out=bass.AP(out_t[:].tensor, 0,
                            [[NC_OUT, P], [P * NC_OUT, NBLK - 1], [1, NC_OUT]]),
                in_=outsb[:, 0:(NBLK - 1) * NC_OUT])
            lastrows = NPC - (NBLK - 1) * P
            nc.sync.dma_start(
                out=bass.AP(out_t[:].tensor, (NBLK - 1) * P * NC_OUT,
                            [[NC_OUT, lastrows], [1, NC_OUT]]),
                in_=outsb[0:lastrows, (NBLK - 1) * NC_OUT:NBLK * NC_OUT])

    nc.compile()
    return nc


def kernel(**inputs):
    x = np.asarray(inputs["x"], np.float32)
    edge_index = np.asarray(inputs["edge_index"])
    plan, data, perm = _host_prep(edge_index)
    Wcat1, Wcat2 = _host_weights(
        inputs["W1"], inputs["a_src1"], inputs["a_dst1"],
        inputs["W2"], inputs["a_src2"], inputs["a_dst2"])
    b1 = np.asarray(inputs["b1"], np.float32).reshape(1, -1)
    b2 = np.asarray(inputs["b2"], np.float32).reshape(1, -1)
    wc = np.asarray(inputs["Wc"], np.float32).reshape(1, -1).copy()
    bc = np.asarray(inputs["bc"], np.float32).reshape(1, -1)

    nc = _build(plan)
    in_maps = []
    for k in range(NCORES):
        in_maps.append({
            "xt": _host_xt(x, perm[k]),
            "w1cat": Wcat1, "w2cat": Wcat2,
            "b1": b1, "b2": b2, "wc": wc, "bc": bc,
            "idxs": data[k]["idx_stream"],
            "dds": data[k]["ddT"],
        })
    res = bass_utils.run_bass_kernel_spmd(
        nc, in_maps, core_ids=list(range(NCORES)),
        trace=globals().get("TRACE", False))
    globals()["LAST_RES"] = res
    outp = np.concatenate([np.asarray(r["out"], np.float32) for r in res.results],
                          axis=0)
    out = np.empty((N_NODES, NC_OUT), np.float32)
    out[perm.reshape(-1)] = outp
    return out


if __name__ == "__main__":
    pass
